# revision 11
# baseline (speedup 1.0000x reference)
"""AMICO ADMM solver on 8 TRN2 NeuronCores.

min_x ||y - A x||^2 + lambda*|x|_1, x >= 0 via ADMM (100 iterations),
data-parallel over voxels (1024 voxels per core).

Reformulation (rho=1, kappa=lambda/rho): with v := x + u and t := v - kappa,
the reference iteration is t' = W@|t| + D + min(t,0), D = W@AtY - kappa*W@1.
Using min(t,0) = t/2 - |t|/2 and folding -|t|/2 into the WEIGHTS:

    t' = (W - I/2) @ s'  +  (I/2) @ (t + 2D)        s' := |t|

Per [128,512] psum region the PE runs [Wt[kc0]@s'; Wt[kc1]@s'; (I/2)@tD]
(12 matmuls/iter, 213 ns each = the 2.56 us/iter floor).  The entire
elementwise budget per iteration is then:
  readout t (PSUM -> fp16 SBUF, each element ONCE):
      ScalarE Copy-act for regions q0,q2; VectorE tensor_copy for q1,q3
  s' = t & 0x7fff   (uint16-bitcast tensor_scalar, 4x mode, 2 wide ops, V)
  tD = t + 2D       (fp16 tensor_tensor add, 2 wide ops, GpSimd -- its only
                     job; SBUF-only so Pool can run it; fallback: VectorE)
There is NO min/mu computation left.  The last iteration uses the original
W weights and injects (I/2)@(2D), so psum == x_100 and the output is a
plain PSUM->SBUF copy.  Iteration 0 injects tD_0 = 2D - kappa (s'_0 = kappa).

Engine budget (trace-calibrated): PE 2.56 us (bottleneck), ScalarE 1.64,
VectorE 2.20, GpSimd 2 wide TT-adds.  Schedule validated by discrete-event
simulation: steady period == PE floor when the GpSimd op <= ~1 us.
"""

import os

import numpy as np

M = 256
K = 256
N_VOX = 8192
N_CORES = 8
N_SHARD = N_VOX // N_CORES  # 1024
RHO = 1.0
LAMBDA_REG = 0.1
KAPPA = LAMBDA_REG / RHO
N_ITERS = 100

USE_GPSIMD = bool(int(os.environ.get("KERNEL_GPSIMD", "1")))

LAST_RESULTS = None  # BassKernelResults of the most recent run (for test.py)

# region q -> (r, c): q0=(0,0) q1=(1,0) q2=(0,1) q3=(1,1)
Q_RC = [(0, 0), (1, 0), (0, 1), (1, 1)]


def _build_graph():
    import concourse.mybir as mybir
    from concourse import bacc
    from concourse.tile import TileContext

    f32 = mybir.dt.float32
    fp16 = mybir.dt.float16
    u16 = mybir.dt.uint16
    kap = float(KAPPA)

    nc = bacc.Bacc("TRN2", target_bir_lowering=False, debug=False)

    # D2[p, q*512 + j] = 2*D[r(q)*128+p, c(q)*512+j]   (host f64 -> fp16)
    D2_p = nc.declare_dram_parameter("D2", [128, 2048], fp16, isOutput=False)
    # Wt16[p, kc*256 + r*128 + j] = (W - I/2)[kc*128+p, r*128+j]
    Wt_p = nc.declare_dram_parameter("Wt16", [128, 512], fp16, isOutput=False)
    # Wo16: original W, used only by the final iteration
    Wo_p = nc.declare_dram_parameter("Wo16", [128, 512], fp16, isOutput=False)
    Ih_p = nc.declare_dram_parameter("identh", [128, 128], fp16, isOutput=False)
    # out[p, q*512 + j] = x[r(q)*128+p, c(q)*512+j]
    O_p = nc.declare_dram_parameter("out", [128, 2048], f32, isOutput=True)

    copy_f = mybir.ActivationFunctionType.Copy
    alu_and = mybir.AluOpType.bitwise_and

    def q_sl(q):
        return slice(q * 512, q * 512 + 512)

    td_engine = "gpsimd" if USE_GPSIMD else "vector"

    with TileContext(nc) as tc:
        with (
            tc.tile_pool(name="static", bufs=1) as statics,
            tc.tile_pool(name="spool", bufs=4) as spool,
            tc.tile_pool(name="tpool", bufs=4) as tpool,
            tc.tile_pool(name="tdpool", bufs=4) as tdpool,
            tc.tile_pool(name="psum_loop", bufs=8, space="PSUM") as psl,
        ):
            D2_sb = statics.tile([128, 2048], fp16, name="D2_sb")
            nc.sync.dma_start(D2_sb[:, :], D2_p[:, :])
            Wt_sb = statics.tile([128, 512], fp16, name="Wt_sb")
            nc.sync.dma_start(Wt_sb[:, :], Wt_p[:, :])
            Wo_sb = statics.tile([128, 512], fp16, name="Wo_sb")
            nc.sync.dma_start(Wo_sb[:, :], Wo_p[:, :])
            ih_sb = statics.tile([128, 128], fp16, name="ih_sb")
            nc.sync.dma_start(ih_sb[:, :], Ih_p[:, :])
            out_sb = statics.tile([128, 2048], f32, name="out_sb")
            # u16 mask 0x7fff tile for the TT-bitvec abs
            mask_sb = statics.tile([128, 512], u16, name="mask_sb")
            nc.vector.memset(mask_sb[:, :], 0x7FFF)
            # Tiny dummy Copy activation so the ACT_TABLE_LOAD overlaps the
            # input DMAs instead of stalling iteration 1.
            warm_sb = statics.tile([1, 8], f32, name="warm_sb")
            nc.vector.memset(warm_sb[:, :], 0.0)
            nc.scalar.activation(
                warm_sb[:, :], warm_sb[:, :], copy_f, bias=0.0, scale=1.0,
            )

            # tD_0 = 2D - kappa (fp16)
            td0_sb = statics.tile([128, 2048], fp16, name="td0_sb")
            nc.vector.tensor_scalar_sub(td0_sb[:, :], D2_sb[:, :], kap)

            # ---- init: s'_0 = kappa (fp16) ----
            # s_q[q]: s' chunk for (kc=r(q), c(q)); rhs of the W-matmuls.
            s_q = []
            for q in range(4):
                s0 = spool.tile([128, 512], fp16, name="s_new", tag="s")
                nc.vector.memset(s0[:, :], kap)
                s_q.append(s0)
            td_q = [None, None, None, None]

            q_of = {rc: q for q, rc in enumerate(Q_RC)}

            # ---- 100 ADMM iterations, fully unrolled ----
            for it in range(N_ITERS):
                last = it == N_ITERS - 1
                W_sb = Wo_sb if last else Wt_sb
                ps_q = []
                for q, (r, c) in enumerate(Q_RC):
                    ps = psl.tile([128, 512], f32, name="ps_t", tag="ps")
                    ps_q.append(ps)
                    for kc in (0, 1):
                        w0 = kc * 256 + r * 128
                        nc.tensor.matmul(
                            ps[:, :],
                            W_sb[:, w0 : w0 + 128],
                            s_q[q_of[(kc, c)]][:, :],
                            start=(kc == 0),
                            stop=False,
                            skip_group_check=True,
                        )
                    if last:
                        inj = D2_sb[:, q_sl(q)]
                    elif it == 0:
                        inj = td0_sb[:, q_sl(q)]
                    else:
                        inj = td_q[q][:, :]
                    nc.tensor.matmul(
                        ps[:, :], ih_sb[:, :], inj,
                        start=False, stop=True, skip_group_check=True,
                    )

                if last:
                    # psum == x_100; copy out on both ScalarE and VectorE.
                    for q in (0, 1):
                        nc.scalar.activation(
                            out_sb[:, q_sl(q)], ps_q[q][:, :], copy_f,
                            bias=0.0, scale=1.0,
                        )
                    for q in (2, 3):
                        nc.vector.tensor_copy(out_sb[:, q_sl(q)], ps_q[q][:, :])
                    for q in range(4):
                        nc.sync.dma_start(O_p[:, q_sl(q)], out_sb[:, q_sl(q)])
                    break

                # ---- t readout: 4 ScalarE Copy acts into SEPARATE tiles so
                # every downstream op waits only its own producer ----
                t_q = []
                for q in range(4):
                    t = tpool.tile([128, 512], fp16, name="t_sb", tag="t")
                    nc.scalar.activation(
                        t[:, :], ps_q[q][:, :], copy_f, bias=0.0, scale=1.0
                    )
                    t_q.append(t)

                # V (narrow fp16 TT ops, 2x mode): interleave abs and td by
                # deadline: abs0, td0, abs1, td1, abs2, abs3.
                new_s = [None, None, None, None]
                new_td = [None, None, None, None]

                def emit_abs(q):
                    sn = spool.tile([128, 512], fp16, name="s_new", tag="s")
                    nc.vector.tensor_tensor(
                        sn[:, :].bitcast(u16), t_q[q][:, :].bitcast(u16),
                        mask_sb[:, :], alu_and,
                    )
                    new_s[q] = sn

                make_td = it < N_ITERS - 2
                emit_abs(0)
                if make_td:
                    td = tdpool.tile([128, 512], fp16, name="td_new", tag="td")
                    nc.vector.tensor_add(td[:, :], t_q[0][:, :], D2_sb[:, q_sl(0)])
                    new_td[0] = td
                emit_abs(1)
                if make_td:
                    td = tdpool.tile([128, 512], fp16, name="td_new", tag="td")
                    nc.vector.tensor_add(td[:, :], t_q[1][:, :], D2_sb[:, q_sl(1)])
                    new_td[1] = td
                emit_abs(2)
                emit_abs(3)
                # G: td for q2, q3 (their inject deadlines are ~2 periods out)
                if make_td:
                    for q in (2, 3):
                        td = tdpool.tile([128, 512], fp16, name=f"td_g{q}", tag=f"tdg{q}")
                        getattr(nc, td_engine).tensor_add(
                            td[:, :], t_q[q][:, :], D2_sb[:, q_sl(q)]
                        )
                        new_td[q] = td

                s_q = new_s
                td_q = new_td

    nc.compile()
    return nc


_GRAPH = None


def kernel(A: np.ndarray, data: np.ndarray) -> np.ndarray:
    global _GRAPH, LAST_RESULTS
    from concourse.bass_utils import run_bass_kernel_spmd

    A = np.ascontiguousarray(np.asarray(A, dtype=np.float32))
    data = np.ascontiguousarray(np.asarray(data, dtype=np.float32))
    assert A.shape == (M, K) and data.shape == (N_VOX, M)

    # Host-side precompute in f64:
    #   W = (AtA + rho I)^-1 (symmetric), D = W@AtY - kappa*(W@1).
    A64 = A.astype(np.float64)
    AtA = A64.T @ A64
    W = np.linalg.inv(AtA + RHO * np.eye(K))
    w1 = KAPPA * (W @ np.ones(K))
    Wt = W - 0.5 * np.eye(K)

    def w_layout(Wm):
        return np.ascontiguousarray(
            Wm.astype(np.float32)
            .reshape(2, 128, K)
            .transpose(1, 0, 2)
            .reshape(128, 2 * K)
            .astype(np.float16)
        )

    Wt_dev = w_layout(Wt)
    Wo_dev = w_layout(W)
    ih_dev = np.ascontiguousarray(0.5 * np.eye(128, dtype=np.float16))

    in_maps = []
    for i in range(N_CORES):
        shard = data[i * N_SHARD : (i + 1) * N_SHARD]  # [1024, 256]
        AtY = A64.T @ shard.astype(np.float64).T  # [256, 1024]
        D = (W @ AtY) - w1[:, None]  # [256, 1024] f64
        D2 = (2.0 * D).astype(np.float16)
        # q-major layout: D2_dev[:, q*512+j] = 2D[r(q)*128+p, c(q)*512+j]
        D2_dev = np.empty((128, 2048), dtype=np.float16)
        for q, (r, c) in enumerate(Q_RC):
            D2_dev[:, q * 512 : (q + 1) * 512] = D2[
                r * 128 : (r + 1) * 128, c * 512 : (c + 1) * 512
            ]
        in_maps.append(
            {
                "D2": np.ascontiguousarray(D2_dev),
                "Wt16": Wt_dev,
                "Wo16": Wo_dev,
                "identh": ih_dev,
            }
        )

    if _GRAPH is None:
        _GRAPH = _build_graph()

    trace = bool(int(os.environ.get("KERNEL_TRACE", "0")))
    res = run_bass_kernel_spmd(
        _GRAPH, in_maps, core_ids=list(range(N_CORES)), trace=trace
    )
    LAST_RESULTS = res

    out = np.empty((N_VOX, K), dtype=np.float32)
    for i in range(N_CORES):
        o = res.results[i]["out"]  # [128, 2048] q-major
        for q, (r, c) in enumerate(Q_RC):
            blk = o[:, q * 512 : (q + 1) * 512]  # x[r*128+p, c*512+j]
            out[i * N_SHARD + c * 512 : i * N_SHARD + c * 512 + 512,
                r * 128 : (r + 1) * 128] = blk.T
    return out
